# revision 9
# baseline (speedup 1.0000x reference)
"""ConvSwiGLU Trainium2 kernel: tensor-parallel over d_ff across 8 NeuronCores.

Layout strategy (all chosen so no on-device transposes are needed):
  - Each core owns a 512-channel slice of d_ff (gate/up columns, conv channels,
    down rows). Every core sees all 8192 tokens.
  - Activations live as [channels(partition), tokens(free)]: the gate/up matmul
    is psum[c, t] = sum_d Wg[d, c] * xT[d, t] with Wg as stored ([d, f]) as the
    stationary operand and x pre-transposed on the host. All matmul operands
    are bf16 (FWL fast-weight-load, cheap LDWEIGHTS).
  - Tokens run in 8 chunks of 1024 (vector-engine per-op overhead is ~200-300ns
    on this part, so wider tiles amortize it; matmuls tile into [128,2,512]
    PSUM bank pairs and are evacuated with a single wide ACT pass).
  - The depthwise conv runs along the free (token) axis as per-partition-scalar
    MAC chains: a DVE tensor_scalar initializes acc = tap0*h (+conv bias on the
    up side), then 4 MACs run either fused on DVE (scalar_tensor_tensor) or as
    ACT mul + GpSimd tensor_tensor add for the chains assigned to the
    otherwise-idle GpSimd engine.
  - Down matmul: psum[m, t] = sum_f Wd[f, m] * hact[f, t] with Wd as stored.
    Partial outputs (yT per core, bf16) are summed on the host (the d_ff
    all-reduce).
  - Two-chunk software pipeline: produce(i) [PE] || conv(i-1) [ACT/DVE/GpSimd]
    || down(i-2) [PE], so down matmuls never wait on the same chunk's conv.
"""

import os
import sys
from contextlib import ExitStack

import ml_dtypes
import numpy as np

for _p in ("/root/.axon_site/_ro/trn_rl_repo", "/opt/trn_rl_repo"):
    if os.path.isdir(_p) and _p not in sys.path:
        sys.path.append(_p)

import concourse.bass as bass
import concourse.tile as tile
from concourse import bacc, mybir
from concourse.bass_utils import run_bass_kernel_spmd

F32 = mybir.dt.float32
BF16 = mybir.dt.bfloat16
AF = mybir.ActivationFunctionType
ALU = mybir.AluOpType

B, L, D = 4, 2048, 1024
F = 4096
NCORES = 8
FS = F // NCORES          # 512 channels per core
KSUB = D // 128           # 8 contraction subtiles for gate/up
GRP = FS // 128           # 4 channel groups per core
MSUB = D // 128           # 8 output row subtiles for down matmul
T = 1024                  # token chunk
TN = T // 512             # 512-col sub-tiles per chunk (PSUM bank size)
TH = T + 4                # chunk + conv halo (last 4 cols filled from next chunk)
NCH = (B * L) // T        # 8 chunks
PER_SEQ = L // T          # 2 chunks per sequence
K = 5                     # conv taps

# Conv MAC chain engine per (g, side): 'v' = fused DVE scalar_tensor_tensor;
# 'p' = ACT mul into tmp + GpSimd tensor_tensor add (GpSimd can't run
# TensorScalarPtr, so its MACs need the mul elsewhere).
CHAIN_ENGINE = {
    (0, 0): "v", (0, 1): "v",
    (1, 0): "v", (1, 1): "v",
    (2, 0): "v", (2, 1): "p",
    (3, 0): "p", (3, 1): "p",
}

_cache = {}


def _build_program():
    """Build + bacc-compile the per-core SPMD Tile program once."""
    nc = bacc.Bacc("TRN2", target_bir_lowering=False, debug=False,
                   enable_asserts=False, num_devices=NCORES)

    xTc = nc.dram_tensor("xTc", [NCH, 128, KSUB, T], BF16, kind="ExternalInput").ap()
    wg = nc.dram_tensor("wgS", [128, KSUB, FS], BF16, kind="ExternalInput").ap()
    wu = nc.dram_tensor("wuS", [128, KSUB, FS], BF16, kind="ExternalInput").ap()
    wd = nc.dram_tensor("wdS", [128, GRP, D], BF16, kind="ExternalInput").ap()
    bg = nc.dram_tensor("bgS", [128, GRP], F32, kind="ExternalInput").ap()
    bu = nc.dram_tensor("buS", [128, GRP], F32, kind="ExternalInput").ap()
    cgw = nc.dram_tensor("cgwS", [128, GRP, K], F32, kind="ExternalInput").ap()
    cuw = nc.dram_tensor("cuwS", [128, GRP, K], F32, kind="ExternalInput").ap()
    cgb = nc.dram_tensor("cgbS", [128, GRP], F32, kind="ExternalInput").ap()
    cub = nc.dram_tensor("cubS", [128, GRP], F32, kind="ExternalInput").ap()
    edge = nc.dram_tensor("edgeS", [128, B, 2, GRP, 4], BF16, kind="ExternalInput").ap()
    yT = nc.dram_tensor("yT", [D, B * L], BF16, kind="ExternalOutput").ap()

    with tile.TileContext(nc) as tc, ExitStack() as ctx:
        consts = ctx.enter_context(tc.tile_pool(name="consts", bufs=1))
        xpool = ctx.enter_context(tc.tile_pool(name="x", bufs=2))
        hpool = ctx.enter_context(tc.tile_pool(name="h", bufs=17))
        accpool = ctx.enter_context(tc.tile_pool(name="acc", bufs=12))
        tmppool = ctx.enter_context(tc.tile_pool(name="tmp", bufs=8))
        gactpool = ctx.enter_context(tc.tile_pool(name="gact", bufs=6))
        hactpool = ctx.enter_context(tc.tile_pool(name="hact", bufs=3))
        outpool = ctx.enter_context(tc.tile_pool(name="out", bufs=2))
        ps_main = ctx.enter_context(tc.tile_pool(name="psm", bufs=2, space="PSUM"))
        ps_dn = ctx.enter_context(tc.tile_pool(name="psd", bufs=2, space="PSUM"))

        # resident weights / constants
        wg_sb = consts.tile([128, KSUB, FS], BF16)
        wu_sb = consts.tile([128, KSUB, FS], BF16)
        wd_sb = consts.tile([128, GRP, D], BF16)
        bg_sb = consts.tile([128, GRP], F32)
        bu_sb = consts.tile([128, GRP], F32)
        cgw_sb = consts.tile([128, GRP, K], F32)
        cuw_sb = consts.tile([128, GRP, K], F32)
        cgb_sb = consts.tile([128, GRP], F32)
        cub_sb = consts.tile([128, GRP], F32)
        edge_sb = consts.tile([128, B, 2, GRP, 4], BF16)
        for sb, dr in ((wg_sb, wg), (wu_sb, wu), (wd_sb, wd), (bg_sb, bg),
                       (bu_sb, bu), (cgw_sb, cgw), (cuw_sb, cuw),
                       (cgb_sb, cgb), (cub_sb, cub), (edge_sb, edge)):
            nc.sync.dma_start(sb[:], dr)

        h_tiles = {}   # chunk -> list of h_sb tiles [128, TH], order (g, side)
        hact_t = {}    # chunk -> hact tile [128, GRP, T]
        acc_t = {}     # chunk -> {(g, ci): acc} for deferred-finish chains
        gact_t = {}    # chunk -> {g: gact} for deferred-finish groups
        xt_t = {}      # chunk -> prefetched xt tile

        def dma_x(i):
            xt = xpool.tile([128, KSUB, T], BF16, tag="xt")
            nc.sync.dma_start(xt[:], xTc[i])
            xt_t[i] = xt

        def produce(i):
            """matmul1 for chunk i -> biased h tiles (cols [0:T))."""
            xt = xt_t.pop(i)
            tiles = []
            for g in range(GRP):
                for w_full, bias_sb in ((wg_sb, bg_sb), (wu_sb, bu_sb)):
                    h_ps = ps_main.tile([128, TN, 512], F32, tag="h_main")
                    for tn in range(TN):
                        for ks in range(KSUB):
                            nc.tensor.matmul(
                                h_ps[:, tn, :],
                                w_full[:, ks, g * 128:(g + 1) * 128],
                                xt[:, ks, tn * 512:(tn + 1) * 512],
                                start=(ks == 0), stop=(ks == KSUB - 1))
                    h_sb = hpool.tile([128, TH], BF16, tag="h_sb")
                    # single wide evac of both PSUM banks, bias fused
                    nc.scalar.activation(h_sb[:, 0:T], h_ps[:], AF.Identity,
                                         bias=bias_sb[:, g:g + 1])
                    # reference zero-pads h at sequence starts: first 2 halo
                    # cols must be 0, not the bias the Identity-copy wrote
                    if i % PER_SEQ == 0:
                        nc.gpsimd.memset(h_sb[:, 0:2], 0.0)
                    tiles.append(h_sb)
            h_tiles[i] = tiles

        def conv_main(i):
            """halo fill + conv MAC chains + fast-side swiglu for chunk i.

            Chains on GpSimd ('p') finish late (serial ~2.5us adds), so their
            silu/final run one iteration later in conv_fin — keeping slow
            dependencies out of the ACT/DVE FIFO heads.
            """
            cur = h_tiles.pop(i)
            nxt = h_tiles.get(i + 1)
            hact = hactpool.tile([128, GRP, T], BF16, tag="hact")
            accs = {}
            # inits first: only depend on this chunk's h (ready last iteration)
            for g in range(GRP):
                for ci, (tapw, tapb) in enumerate(((cgw_sb, None),
                                                   (cuw_sb, cub_sb))):
                    acc = accpool.tile([128, T], BF16, tag="acc")
                    h_sb = cur[2 * g + ci]
                    if tapb is None:
                        nc.vector.tensor_scalar(acc[:], h_sb[:, 0:T],
                                                tapw[:, g, 0:1], None, ALU.mult)
                    else:
                        nc.vector.tensor_scalar(acc[:], h_sb[:, 0:T],
                                                tapw[:, g, 0:1], tapb[:, g:g + 1],
                                                ALU.mult, ALU.add)
                    accs[(g, ci)] = acc
            # halo fills (wait on first cols of next chunk's h evacs)
            for g in range(GRP):
                for ci in range(2):
                    h_sb = cur[2 * g + ci]
                    if i % PER_SEQ == PER_SEQ - 1:
                        # tokens L-2, L-1 are in no chunk's main region; their
                        # h (host-computed) + 2 zero-pad cols come via edgeS
                        nc.vector.tensor_copy(h_sb[:, T:TH],
                                              edge_sb[:, i // PER_SEQ, ci, g, :])
                    else:
                        nc.vector.tensor_copy(h_sb[:, T:TH],
                                              nxt[2 * g + ci][:, 0:4])
            # MAC chains
            for g in range(GRP):
                for ci, tapw in enumerate((cgw_sb, cuw_sb)):
                    h_sb = cur[2 * g + ci]
                    acc = accs[(g, ci)]
                    if CHAIN_ENGINE[(g, ci)] == "v":
                        for j in range(1, K):
                            nc.vector.scalar_tensor_tensor(
                                acc[:], h_sb[:, j:j + T], tapw[:, g, j:j + 1],
                                acc[:], ALU.mult, ALU.add)
                    else:
                        tmps = []
                        for j in range(1, K):
                            tmp = tmppool.tile([128, T], BF16, tag="tmp")
                            nc.scalar.mul(tmp[:], h_sb[:, j:j + T],
                                          tapw[:, g, j:j + 1])
                            tmps.append(tmp)
                        for tmp in tmps:
                            nc.gpsimd.tensor_tensor(acc[:], tmp[:], acc[:],
                                                    ALU.add)
            # fast-side swiglu: groups whose chains both ran on DVE
            gacts = {}
            for g in range(GRP):
                if CHAIN_ENGINE[(g, 0)] == "v":
                    gact = gactpool.tile([128, T], BF16, tag="gact")
                    nc.scalar.activation(gact[:], accs[(g, 0)][:], AF.Silu,
                                         bias=cgb_sb[:, g:g + 1])
                    gacts[g] = gact
                    if CHAIN_ENGINE[(g, 1)] == "v":
                        nc.vector.tensor_tensor(hact[:, g, :], accs[(g, 1)][:],
                                                gact[:], ALU.mult)
            hact_t[i] = hact
            acc_t[i] = accs
            gact_t[i] = gacts

        def conv_fin(i):
            """swiglu for chunk i's GpSimd chains (their accs are ready now)."""
            hact = hact_t[i]
            accs = acc_t.pop(i)
            gacts = gact_t.pop(i)
            for g in range(GRP):
                if CHAIN_ENGINE[(g, 0)] == "v" and CHAIN_ENGINE[(g, 1)] == "v":
                    continue
                if g in gacts:
                    gact = gacts[g]
                else:
                    gact = gactpool.tile([128, T], BF16, tag="gact")
                    nc.scalar.activation(gact[:], accs[(g, 0)][:], AF.Silu,
                                         bias=cgb_sb[:, g:g + 1])
                nc.gpsimd.tensor_tensor(hact[:, g, :], accs[(g, 1)][:],
                                        gact[:], ALU.mult)

        def down(i):
            """down matmul + evac + store for chunk i."""
            hact = hact_t.pop(i)
            out_sb = outpool.tile([128, MSUB, T], BF16, tag="out")
            for ms in range(MSUB):
                dn_ps = ps_dn.tile([128, TN, 512], F32, tag="dn")
                for tn in range(TN):
                    for g in range(GRP):
                        nc.tensor.matmul(
                            dn_ps[:, tn, :],
                            wd_sb[:, g, ms * 128:(ms + 1) * 128],
                            hact[:, g, tn * 512:(tn + 1) * 512],
                            start=(g == 0), stop=(g == GRP - 1))
                nc.scalar.copy(out_sb[:, ms, :], dn_ps[:])
            nc.sync.dma_start(
                yT.rearrange("(ms p) t -> p ms t", p=128)[:, :, i * T:(i + 1) * T],
                out_sb[:])

        dma_x(0)
        for i in range(NCH + 3):
            if i + 1 < NCH:
                dma_x(i + 1)
            if 0 <= i - 3:
                down(i - 3)
            if i < NCH:
                produce(i)
            if 0 <= i - 2 < NCH:
                conv_fin(i - 2)
            if 0 <= i - 1 < NCH:
                conv_main(i - 1)

    nc.compile()
    return nc


def _prep_inputs(x, Wg, bgv, Wu, buv, convg_w, convg_b, convu_w, convu_b, Wd):
    """Host-side shard/layout. Returns list of per-core in_maps."""
    x = np.ascontiguousarray(x, np.float32)
    # padded transpose: [B, D, L+4] with zero halo at sequence edges; chunk j
    # of a sequence is cols [T*j, T*j+T) = tokens [T*j-2, T*j+T-2)
    xp = np.zeros((B, D, L + 4), np.float32)
    xp[:, :, 2:L + 2] = x.transpose(0, 2, 1)
    blocks = np.stack([xp[:, :, T * j:T * j + T] for j in range(PER_SEQ)], axis=1)
    xTc = np.ascontiguousarray(
        blocks.reshape(NCH, KSUB, 128, T).transpose(0, 2, 1, 3)).astype(
            ml_dtypes.bfloat16)

    def colsplit(w, c):      # [D, F] -> per-core [128, KSUB, FS] bf16
        s = w[:, c * FS:(c + 1) * FS]
        return np.ascontiguousarray(
            s.reshape(KSUB, 128, FS).transpose(1, 0, 2)).astype(ml_dtypes.bfloat16)

    def vecsplit(v, c):      # [F] -> [128, GRP]
        return np.ascontiguousarray(v[c * FS:(c + 1) * FS].reshape(GRP, 128).T)

    xe = np.asarray(x, np.float32)[:, L - 2:L, :]          # [B, 2, D]
    in_maps = []
    for c in range(NCORES):
        sl = slice(c * FS, (c + 1) * FS)
        he_g = xe @ np.asarray(Wg, np.float32)[:, sl] + np.asarray(bgv, np.float32)[sl]
        he_u = xe @ np.asarray(Wu, np.float32)[:, sl] + np.asarray(buv, np.float32)[sl]
        edgeS = np.zeros((128, B, 2, GRP, 4), np.float32)
        for s in range(B):
            for ci, he in enumerate((he_g, he_u)):
                # [2, FS] -> [128, GRP, 2] at halo cols 0,1 (tokens L-2, L-1)
                v = he[s].reshape(2, GRP, 128).transpose(2, 1, 0)
                edgeS[:, s, ci, :, 0:2] = v
        edgeS = edgeS.astype(ml_dtypes.bfloat16)
        wdS = Wd[c * FS:(c + 1) * FS, :]
        in_maps.append({
            "edgeS": edgeS,
            "xTc": xTc,
            "wgS": colsplit(np.asarray(Wg, np.float32), c),
            "wuS": colsplit(np.asarray(Wu, np.float32), c),
            "wdS": np.ascontiguousarray(
                np.asarray(wdS, np.float32).reshape(GRP, 128, D).transpose(1, 0, 2)
                .astype(ml_dtypes.bfloat16)),
            "bgS": vecsplit(np.asarray(bgv, np.float32), c),
            "buS": vecsplit(np.asarray(buv, np.float32), c),
            "cgwS": np.ascontiguousarray(
                np.asarray(convg_w, np.float32)[c * FS:(c + 1) * FS, 0, :]
                .reshape(GRP, 128, K).transpose(1, 0, 2)),
            "cuwS": np.ascontiguousarray(
                np.asarray(convu_w, np.float32)[c * FS:(c + 1) * FS, 0, :]
                .reshape(GRP, 128, K).transpose(1, 0, 2)),
            "cgbS": vecsplit(np.asarray(convg_b, np.float32), c),
            "cubS": vecsplit(np.asarray(convu_b, np.float32), c),
        })
    return in_maps


def run_on_cores(in_maps, **kwargs):
    if "nc" not in _cache:
        _cache["nc"] = _build_program()
    return run_bass_kernel_spmd(_cache["nc"], in_maps,
                                core_ids=list(range(NCORES)), **kwargs)


def kernel(x, Wg, bg, Wu, bu, convg_w, convg_b, convu_w, convu_b, Wd, bd):
    in_maps = _prep_inputs(x, Wg, bg, Wu, bu, convg_w, convg_b,
                           convu_w, convu_b, Wd)
    res = run_on_cores(in_maps)
    acc = np.zeros((D, B * L), np.float64)
    for r in res.results:
        acc += np.asarray(r["yT"], np.float64)
    acc += np.asarray(bd, np.float64)[:, None]
    return np.ascontiguousarray(acc.T.reshape(B, L, D)).astype(np.float32)


# revision 12
# speedup vs baseline: 1.3234x; 1.3234x over previous
"""ConvSwiGLU Trainium2 kernel: tensor-parallel over d_ff across 8 NeuronCores.

Layout strategy (all chosen so no on-device transposes are needed):
  - Each core owns a 512-channel slice of d_ff (gate/up columns, conv channels,
    down rows). Every core sees all 8192 tokens.
  - Activations live as [channels(partition), tokens(free)]: the gate/up matmul
    is psum[c, t] = sum_d Wg[d, c] * xT[d, t] with Wg as stored ([d, f]) as the
    stationary operand and x pre-transposed on the host. All matmul operands
    are bf16 (FWL fast-weight-load, cheap LDWEIGHTS).
  - Tokens run in 8 chunks of 1024 (vector-engine per-op overhead is ~200-300ns
    on this part, so wider tiles amortize it; matmuls tile into [128,2,512]
    PSUM bank pairs and are evacuated with a single wide ACT pass).
  - The depthwise conv runs along the free (token) axis as per-partition-scalar
    MAC chains: a DVE tensor_scalar initializes acc = tap0*h (+conv bias on the
    up side), then 4 MACs run either fused on DVE (scalar_tensor_tensor) or as
    ACT mul + GpSimd tensor_tensor add for the chains assigned to the
    otherwise-idle GpSimd engine.
  - Down matmul: psum[m, t] = sum_f Wd[f, m] * hact[f, t] with Wd as stored.
    Partial outputs (yT per core, bf16) are summed on the host (the d_ff
    all-reduce).
  - Two-chunk software pipeline: produce(i) [PE] || conv(i-1) [ACT/DVE/GpSimd]
    || down(i-2) [PE], so down matmuls never wait on the same chunk's conv.
"""

import os
import sys
from contextlib import ExitStack

import ml_dtypes
import numpy as np

for _p in ("/root/.axon_site/_ro/trn_rl_repo", "/opt/trn_rl_repo"):
    if os.path.isdir(_p) and _p not in sys.path:
        sys.path.append(_p)

import concourse.bass as bass
import concourse.tile as tile
from concourse import bacc, mybir
from concourse.bass_utils import run_bass_kernel_spmd

F32 = mybir.dt.float32
BF16 = mybir.dt.bfloat16
AF = mybir.ActivationFunctionType
ALU = mybir.AluOpType

B, L, D = 4, 2048, 1024
F = 4096
NCORES = 8
FS = F // NCORES          # 512 channels per core
KSUB = D // 128           # 8 contraction subtiles for gate/up
GRP = FS // 128           # 4 channel groups per core
MSUB = D // 128           # 8 output row subtiles for down matmul
T = 1024                  # token chunk
TN = T // 512             # 512-col sub-tiles per chunk (PSUM bank size)
TH = T + 4                # chunk + conv halo (last 4 cols filled from next chunk)
NCH = (B * L) // T        # 8 chunks
PER_SEQ = L // T          # 2 chunks per sequence
K = 5                     # conv taps

# Conv MAC form per (g, side) -> 4 slots (taps 1-4): 'v' = fused DVE
# scalar_tensor_tensor (~1.3us, no fast mode); 'a' = ACT mul into tmp
# (~1.16us) + DVE tensor_tensor add (2x_1p mode, ~0.72us).  GpSimd is
# deliberately unused for compute: any GpSimd op locks the SBUF port pair
# it shares with DVE, fully blocking DVE's 2-port fast-mode ops.
CHAIN_SLOTS = {
    (0, 0): "vvvv",
    (0, 1): "vvvv",
    (1, 0): "vvvv",
    (1, 1): "vvvv",
    (2, 0): "vvva",
    (2, 1): "aaaa",
    (3, 0): "aaaa",
    (3, 1): "aaaa",
}

_cache = {}


def _build_program():
    """Build + bacc-compile the per-core SPMD Tile program once."""
    nc = bacc.Bacc("TRN2", target_bir_lowering=False, debug=False,
                   enable_asserts=False, num_devices=NCORES)

    xTc = nc.dram_tensor("xTc", [NCH, 128, KSUB, T], BF16, kind="ExternalInput").ap()
    wg = nc.dram_tensor("wgS", [128, KSUB, FS], BF16, kind="ExternalInput").ap()
    wu = nc.dram_tensor("wuS", [128, KSUB, FS], BF16, kind="ExternalInput").ap()
    wd = nc.dram_tensor("wdS", [128, GRP, D], BF16, kind="ExternalInput").ap()
    bg = nc.dram_tensor("bgS", [128, GRP], F32, kind="ExternalInput").ap()
    bu = nc.dram_tensor("buS", [128, GRP], F32, kind="ExternalInput").ap()
    cgw = nc.dram_tensor("cgwS", [128, GRP, K], F32, kind="ExternalInput").ap()
    cuw = nc.dram_tensor("cuwS", [128, GRP, K], F32, kind="ExternalInput").ap()
    cgb = nc.dram_tensor("cgbS", [128, GRP], F32, kind="ExternalInput").ap()
    cub = nc.dram_tensor("cubS", [128, GRP], F32, kind="ExternalInput").ap()
    edge = nc.dram_tensor("edgeS", [128, B, 2, GRP, 4], BF16, kind="ExternalInput").ap()
    yT = nc.dram_tensor("yT", [D, B * L], BF16, kind="ExternalOutput").ap()

    with tile.TileContext(nc) as tc, ExitStack() as ctx:
        consts = ctx.enter_context(tc.tile_pool(name="consts", bufs=1))
        xpool = ctx.enter_context(tc.tile_pool(name="x", bufs=2))
        hpool = ctx.enter_context(tc.tile_pool(name="h", bufs=17))
        accpool = ctx.enter_context(tc.tile_pool(name="acc", bufs=12))
        tmppool = ctx.enter_context(tc.tile_pool(name="tmp", bufs=8))
        gactpool = ctx.enter_context(tc.tile_pool(name="gact", bufs=6))
        hactpool = ctx.enter_context(tc.tile_pool(name="hact", bufs=3))
        outpool = ctx.enter_context(tc.tile_pool(name="out", bufs=2))
        ps_main = ctx.enter_context(tc.tile_pool(name="psm", bufs=2, space="PSUM"))
        ps_dn = ctx.enter_context(tc.tile_pool(name="psd", bufs=2, space="PSUM"))

        # resident weights / constants
        wg_sb = consts.tile([128, KSUB, FS], BF16)
        wu_sb = consts.tile([128, KSUB, FS], BF16)
        wd_sb = consts.tile([128, GRP, D], BF16)
        bg_sb = consts.tile([128, GRP], F32)
        bu_sb = consts.tile([128, GRP], F32)
        cgw_sb = consts.tile([128, GRP, K], F32)
        cuw_sb = consts.tile([128, GRP, K], F32)
        cgb_sb = consts.tile([128, GRP], F32)
        cub_sb = consts.tile([128, GRP], F32)
        edge_sb = consts.tile([128, B, 2, GRP, 4], BF16)
        for sb, dr in ((wg_sb, wg), (wu_sb, wu), (wd_sb, wd), (bg_sb, bg),
                       (bu_sb, bu), (cgw_sb, cgw), (cuw_sb, cuw),
                       (cgb_sb, cgb), (cub_sb, cub), (edge_sb, edge)):
            nc.sync.dma_start(sb[:], dr)

        h_tiles = {}   # chunk -> list of h_sb tiles [128, TH], order (g, side)
        hact_t = {}    # chunk -> hact tile [128, GRP, T]
        acc_t = {}     # chunk -> {(g, ci): acc} for deferred-finish chains
        gact_t = {}    # chunk -> {g: gact} for deferred-finish groups
        xt_t = {}      # chunk -> prefetched xt tile

        def dma_x(i):
            xt = xpool.tile([128, KSUB, T], BF16, tag="xt")
            nc.sync.dma_start(xt[:], xTc[i])
            xt_t[i] = xt

        def produce(i):
            """matmul1 for chunk i -> biased h tiles (cols [0:T))."""
            xt = xt_t.pop(i)
            tiles = []
            for g in range(GRP):
                for w_full, bias_sb in ((wg_sb, bg_sb), (wu_sb, bu_sb)):
                    h_ps = ps_main.tile([128, TN, 512], F32, tag="h_main")
                    for tn in range(TN):
                        for ks in range(KSUB):
                            nc.tensor.matmul(
                                h_ps[:, tn, :],
                                w_full[:, ks, g * 128:(g + 1) * 128],
                                xt[:, ks, tn * 512:(tn + 1) * 512],
                                start=(ks == 0), stop=(ks == KSUB - 1))
                    h_sb = hpool.tile([128, TH], BF16, tag="h_sb")
                    # single wide evac of both PSUM banks, bias fused
                    nc.scalar.activation(h_sb[:, 0:T], h_ps[:], AF.Identity,
                                         bias=bias_sb[:, g:g + 1])
                    # reference zero-pads h at sequence starts: first 2 halo
                    # cols must be 0, not the bias the Identity-copy wrote
                    if i % PER_SEQ == 0:
                        nc.gpsimd.memset(h_sb[:, 0:2], 0.0)
                    tiles.append(h_sb)
            h_tiles[i] = tiles

        def conv_main(i):
            """halo fill + conv MAC chains + swiglu for chunk i -> hact."""
            cur = h_tiles.pop(i)
            nxt = h_tiles.get(i + 1)
            hact = hactpool.tile([128, GRP, T], BF16, tag="hact")
            accs = {}
            # inits first: only depend on this chunk's h (ready last iteration)
            for g in range(GRP):
                for ci, (tapw, tapb) in enumerate(((cgw_sb, None),
                                                   (cuw_sb, cub_sb))):
                    acc = accpool.tile([128, T], BF16, tag="acc")
                    h_sb = cur[2 * g + ci]
                    if tapb is None:
                        nc.vector.tensor_scalar(acc[:], h_sb[:, 0:T],
                                                tapw[:, g, 0:1], None, ALU.mult)
                    else:
                        nc.vector.tensor_scalar(acc[:], h_sb[:, 0:T],
                                                tapw[:, g, 0:1], tapb[:, g:g + 1],
                                                ALU.mult, ALU.add)
                    accs[(g, ci)] = acc
            # halo fills (wait on first cols of next chunk's h evacs)
            for g in range(GRP):
                for ci in range(2):
                    h_sb = cur[2 * g + ci]
                    if i % PER_SEQ == PER_SEQ - 1:
                        # tokens L-2, L-1 are in no chunk's main region; their
                        # h (host-computed) + 2 zero-pad cols come via edgeS
                        nc.vector.tensor_copy(h_sb[:, T:TH],
                                              edge_sb[:, i // PER_SEQ, ci, g, :])
                    else:
                        nc.vector.tensor_copy(h_sb[:, T:TH],
                                              nxt[2 * g + ci][:, 0:4])
            # MAC chains: ACT muls for 'a' slots can all start immediately;
            # the serial part per chain is stt / tt adds on DVE
            for g in range(GRP):
                for ci, tapw in enumerate((cgw_sb, cuw_sb)):
                    h_sb = cur[2 * g + ci]
                    acc = accs[(g, ci)]
                    slots = CHAIN_SLOTS[(g, ci)]
                    tmps = {}
                    for j in range(1, K):
                        if slots[j - 1] == "a":
                            tmp = tmppool.tile([128, T], BF16, tag="tmp")
                            nc.scalar.mul(tmp[:], h_sb[:, j:j + T],
                                          tapw[:, g, j:j + 1])
                            tmps[j] = tmp
                    for j in range(1, K):
                        if slots[j - 1] == "v":
                            nc.vector.scalar_tensor_tensor(
                                acc[:], h_sb[:, j:j + T], tapw[:, g, j:j + 1],
                                acc[:], ALU.mult, ALU.add)
                        else:
                            nc.vector.tensor_tensor(acc[:], tmps[j][:], acc[:],
                                                    ALU.add)
            # swiglu
            for g in range(GRP):
                gact = gactpool.tile([128, T], BF16, tag="gact")
                nc.scalar.activation(gact[:], accs[(g, 0)][:], AF.Silu,
                                     bias=cgb_sb[:, g:g + 1])
                nc.vector.tensor_tensor(hact[:, g, :], accs[(g, 1)][:],
                                        gact[:], ALU.mult)
            hact_t[i] = hact

        def down(i):
            """down matmul + evac + store for chunk i."""
            hact = hact_t.pop(i)
            out_sb = outpool.tile([128, MSUB, T], BF16, tag="out")
            for ms in range(MSUB):
                dn_ps = ps_dn.tile([128, TN, 512], F32, tag="dn")
                for tn in range(TN):
                    for g in range(GRP):
                        nc.tensor.matmul(
                            dn_ps[:, tn, :],
                            wd_sb[:, g, ms * 128:(ms + 1) * 128],
                            hact[:, g, tn * 512:(tn + 1) * 512],
                            start=(g == 0), stop=(g == GRP - 1))
                nc.scalar.copy(out_sb[:, ms, :], dn_ps[:])
            nc.sync.dma_start(
                yT.rearrange("(ms p) t -> p ms t", p=128)[:, :, i * T:(i + 1) * T],
                out_sb[:])

        dma_x(0)
        for i in range(NCH + 2):
            if i + 1 < NCH:
                dma_x(i + 1)
            if 0 <= i - 2:
                down(i - 2)
            if i < NCH:
                produce(i)
            if 0 <= i - 1 < NCH:
                conv_main(i - 1)

    nc.compile()
    return nc


def _prep_inputs(x, Wg, bgv, Wu, buv, convg_w, convg_b, convu_w, convu_b, Wd):
    """Host-side shard/layout. Returns list of per-core in_maps."""
    x = np.ascontiguousarray(x, np.float32)
    # padded transpose: [B, D, L+4] with zero halo at sequence edges; chunk j
    # of a sequence is cols [T*j, T*j+T) = tokens [T*j-2, T*j+T-2)
    xp = np.zeros((B, D, L + 4), np.float32)
    xp[:, :, 2:L + 2] = x.transpose(0, 2, 1)
    blocks = np.stack([xp[:, :, T * j:T * j + T] for j in range(PER_SEQ)], axis=1)
    xTc = np.ascontiguousarray(
        blocks.reshape(NCH, KSUB, 128, T).transpose(0, 2, 1, 3)).astype(
            ml_dtypes.bfloat16)

    def colsplit(w, c):      # [D, F] -> per-core [128, KSUB, FS] bf16
        s = w[:, c * FS:(c + 1) * FS]
        return np.ascontiguousarray(
            s.reshape(KSUB, 128, FS).transpose(1, 0, 2)).astype(ml_dtypes.bfloat16)

    def vecsplit(v, c):      # [F] -> [128, GRP]
        return np.ascontiguousarray(v[c * FS:(c + 1) * FS].reshape(GRP, 128).T)

    xe = np.asarray(x, np.float32)[:, L - 2:L, :]          # [B, 2, D]
    in_maps = []
    for c in range(NCORES):
        sl = slice(c * FS, (c + 1) * FS)
        he_g = xe @ np.asarray(Wg, np.float32)[:, sl] + np.asarray(bgv, np.float32)[sl]
        he_u = xe @ np.asarray(Wu, np.float32)[:, sl] + np.asarray(buv, np.float32)[sl]
        edgeS = np.zeros((128, B, 2, GRP, 4), np.float32)
        for s in range(B):
            for ci, he in enumerate((he_g, he_u)):
                # [2, FS] -> [128, GRP, 2] at halo cols 0,1 (tokens L-2, L-1)
                v = he[s].reshape(2, GRP, 128).transpose(2, 1, 0)
                edgeS[:, s, ci, :, 0:2] = v
        edgeS = edgeS.astype(ml_dtypes.bfloat16)
        wdS = Wd[c * FS:(c + 1) * FS, :]
        in_maps.append({
            "edgeS": edgeS,
            "xTc": xTc,
            "wgS": colsplit(np.asarray(Wg, np.float32), c),
            "wuS": colsplit(np.asarray(Wu, np.float32), c),
            "wdS": np.ascontiguousarray(
                np.asarray(wdS, np.float32).reshape(GRP, 128, D).transpose(1, 0, 2)
                .astype(ml_dtypes.bfloat16)),
            "bgS": vecsplit(np.asarray(bgv, np.float32), c),
            "buS": vecsplit(np.asarray(buv, np.float32), c),
            "cgwS": np.ascontiguousarray(
                np.asarray(convg_w, np.float32)[c * FS:(c + 1) * FS, 0, :]
                .reshape(GRP, 128, K).transpose(1, 0, 2)),
            "cuwS": np.ascontiguousarray(
                np.asarray(convu_w, np.float32)[c * FS:(c + 1) * FS, 0, :]
                .reshape(GRP, 128, K).transpose(1, 0, 2)),
            "cgbS": vecsplit(np.asarray(convg_b, np.float32), c),
            "cubS": vecsplit(np.asarray(convu_b, np.float32), c),
        })
    return in_maps


def run_on_cores(in_maps, **kwargs):
    if "nc" not in _cache:
        _cache["nc"] = _build_program()
    return run_bass_kernel_spmd(_cache["nc"], in_maps,
                                core_ids=list(range(NCORES)), **kwargs)


def kernel(x, Wg, bg, Wu, bu, convg_w, convg_b, convu_w, convu_b, Wd, bd):
    in_maps = _prep_inputs(x, Wg, bg, Wu, bu, convg_w, convg_b,
                           convu_w, convu_b, Wd)
    res = run_on_cores(in_maps)
    acc = np.zeros((D, B * L), np.float64)
    for r in res.results:
        acc += np.asarray(r["yT"], np.float64)
    acc += np.asarray(bd, np.float64)[:, None]
    return np.ascontiguousarray(acc.T.reshape(B, L, D)).astype(np.float32)


# revision 16
# speedup vs baseline: 1.3450x; 1.0163x over previous
"""ConvSwiGLU Trainium2 kernel: tensor-parallel over d_ff across 8 NeuronCores.

Layout strategy (all chosen so no on-device transposes are needed):
  - Each core owns a 512-channel slice of d_ff (gate/up columns, conv channels,
    down rows). Every core sees all 8192 tokens.
  - Activations live as [channels(partition), tokens(free)]: the gate/up matmul
    is psum[c, t] = sum_d Wg[d, c] * xT[d, t] with Wg as stored ([d, f]) as the
    stationary operand and x pre-transposed on the host. All matmul operands
    are bf16 (FWL fast-weight-load, cheap LDWEIGHTS).
  - Tokens run in 8 chunks of 1024 (vector-engine per-op overhead is ~200-300ns
    on this part, so wider tiles amortize it; matmuls tile into [128,2,512]
    PSUM bank pairs and are evacuated with a single wide ACT pass).
  - The depthwise conv runs along the free (token) axis as per-partition-scalar
    MAC chains: a DVE tensor_scalar initializes acc = tap0*h (+conv bias on the
    up side), then 4 MACs run either fused on DVE (scalar_tensor_tensor) or as
    ACT mul + GpSimd tensor_tensor add for the chains assigned to the
    otherwise-idle GpSimd engine.
  - Down matmul: psum[m, t] = sum_f Wd[f, m] * hact[f, t] with Wd as stored.
    Partial outputs (yT per core, bf16) are summed on the host (the d_ff
    all-reduce).
  - Two-chunk software pipeline: produce(i) [PE] || conv(i-1) [ACT/DVE/GpSimd]
    || down(i-2) [PE], so down matmuls never wait on the same chunk's conv.
"""

import os
import sys
from contextlib import ExitStack

import ml_dtypes
import numpy as np

for _p in ("/root/.axon_site/_ro/trn_rl_repo", "/opt/trn_rl_repo"):
    if os.path.isdir(_p) and _p not in sys.path:
        sys.path.append(_p)

import concourse.bass as bass
import concourse.tile as tile
from concourse import bacc, mybir
from concourse.bass_utils import run_bass_kernel_spmd

F32 = mybir.dt.float32
BF16 = mybir.dt.bfloat16
AF = mybir.ActivationFunctionType
ALU = mybir.AluOpType

B, L, D = 4, 2048, 1024
F = 4096
NCORES = 8
FS = F // NCORES          # 512 channels per core
KSUB = D // 128           # 8 contraction subtiles for gate/up
GRP = FS // 128           # 4 channel groups per core
MSUB = D // 128           # 8 output row subtiles for down matmul
T = 1024                  # token chunk
TN = T // 512             # 512-col sub-tiles per chunk (PSUM bank size)
TH = T + 4                # chunk + conv halo (last 4 cols filled from next chunk)
NCH = (B * L) // T        # 8 chunks
PER_SEQ = L // T          # 2 chunks per sequence
K = 5                     # conv taps

# Conv MAC form per (g, side) -> 4 slots (taps 1-4): 'v' = fused DVE
# scalar_tensor_tensor (~1.3us, no fast mode); 'a' = ACT mul into tmp
# (~1.16us) + DVE tensor_tensor add (2x_1p mode, ~0.72us).  GpSimd is
# deliberately unused for compute: any GpSimd op locks the SBUF port pair
# it shares with DVE, fully blocking DVE's 2-port fast-mode ops.
CHAIN_SLOTS = {
    (0, 0): "vvvv",
    (0, 1): "vvvv",
    (1, 0): "vvvv",
    (1, 1): "vvvv",
    (2, 0): "vvva",
    (2, 1): "aaaa",
    (3, 0): "aaaa",
    (3, 1): "aaaa",
}

_cache = {}


def _build_program():
    """Build + bacc-compile the per-core SPMD Tile program once."""
    nc = bacc.Bacc("TRN2", target_bir_lowering=False, debug=False,
                   enable_asserts=False, num_devices=NCORES)

    xTc = nc.dram_tensor("xTc", [NCH, 128, KSUB, T], BF16, kind="ExternalInput").ap()
    wg = nc.dram_tensor("wgS", [128, KSUB, FS], BF16, kind="ExternalInput").ap()
    wu = nc.dram_tensor("wuS", [128, KSUB, FS], BF16, kind="ExternalInput").ap()
    wd = nc.dram_tensor("wdS", [128, GRP, D], BF16, kind="ExternalInput").ap()
    bg = nc.dram_tensor("bgS", [128, GRP], F32, kind="ExternalInput").ap()
    bu = nc.dram_tensor("buS", [128, GRP], F32, kind="ExternalInput").ap()
    cgw = nc.dram_tensor("cgwS", [128, GRP, K], F32, kind="ExternalInput").ap()
    cuw = nc.dram_tensor("cuwS", [128, GRP, K], F32, kind="ExternalInput").ap()
    cgb = nc.dram_tensor("cgbS", [128, GRP], F32, kind="ExternalInput").ap()
    cub = nc.dram_tensor("cubS", [128, GRP], F32, kind="ExternalInput").ap()
    edge = nc.dram_tensor("edgeS", [128, B, 2, GRP, 4], BF16, kind="ExternalInput").ap()
    yT = nc.dram_tensor("yT", [D, B * L], BF16, kind="ExternalOutput").ap()

    with tile.TileContext(nc) as tc, ExitStack() as ctx:
        consts = ctx.enter_context(tc.tile_pool(name="consts", bufs=1))
        xpool = ctx.enter_context(tc.tile_pool(name="x", bufs=2))
        hpool = ctx.enter_context(tc.tile_pool(name="h", bufs=17))
        accpool = ctx.enter_context(tc.tile_pool(name="acc", bufs=12))
        tmppool = ctx.enter_context(tc.tile_pool(name="tmp", bufs=8))
        gactpool = ctx.enter_context(tc.tile_pool(name="gact", bufs=6))
        hactpool = ctx.enter_context(tc.tile_pool(name="hact", bufs=3))
        outpool = ctx.enter_context(tc.tile_pool(name="out", bufs=2))
        ps_main = ctx.enter_context(tc.tile_pool(name="psm", bufs=2, space="PSUM"))
        ps_dn = ctx.enter_context(tc.tile_pool(name="psd", bufs=2, space="PSUM"))

        # resident weights / constants
        wg_sb = consts.tile([128, KSUB, FS], BF16)
        wu_sb = consts.tile([128, KSUB, FS], BF16)
        wd_sb = consts.tile([128, GRP, D], BF16)
        bg_sb = consts.tile([128, GRP], F32)
        bu_sb = consts.tile([128, GRP], F32)
        cgw_sb = consts.tile([128, GRP, K], F32)
        cuw_sb = consts.tile([128, GRP, K], F32)
        cgb_sb = consts.tile([128, GRP], F32)
        cub_sb = consts.tile([128, GRP], F32)
        edge_sb = consts.tile([128, B, 2, GRP, 4], BF16)
        # gate/up weights + biases first: produce(0) needs them immediately;
        # everything else is first read ~40us in (conv(0)/down(0))
        for sb, dr in ((wg_sb, wg), (wu_sb, wu), (bg_sb, bg), (bu_sb, bu)):
            nc.sync.dma_start(sb[:], dr)
        late_consts = ((wd_sb, wd), (cgw_sb, cgw), (cuw_sb, cuw),
                       (cgb_sb, cgb), (cub_sb, cub), (edge_sb, edge))

        h_tiles = {}   # chunk -> list of h_sb tiles [128, TH], order (g, side)
        hact_t = {}    # chunk -> hact tile [128, GRP, T]
        acc_t = {}     # chunk -> {(g, ci): acc} for deferred-finish chains
        gact_t = {}    # chunk -> {g: gact} for deferred-finish groups
        xt_t = {}      # chunk -> prefetched xt tile

        def dma_x(i):
            xt = xpool.tile([128, KSUB, T], BF16, tag="xt")
            nc.sync.dma_start(xt[:], xTc[i])
            xt_t[i] = xt

        def produce(i):
            """matmul1 for chunk i -> biased h tiles (cols [0:T))."""
            xt = xt_t.pop(i)
            tiles = []
            for g in range(GRP):
                for w_full, bias_sb in ((wg_sb, bg_sb), (wu_sb, bu_sb)):
                    h_ps = ps_main.tile([128, TN, 512], F32, tag="h_main")
                    for tn in range(TN):
                        for ks in range(KSUB):
                            nc.tensor.matmul(
                                h_ps[:, tn, :],
                                w_full[:, ks, g * 128:(g + 1) * 128],
                                xt[:, ks, tn * 512:(tn + 1) * 512],
                                start=(ks == 0), stop=(ks == KSUB - 1))
                    h_sb = hpool.tile([128, TH], BF16, tag="h_sb")
                    # single wide evac of both PSUM banks, bias fused
                    nc.scalar.activation(h_sb[:, 0:T], h_ps[:], AF.Identity,
                                         bias=bias_sb[:, g:g + 1])
                    # reference zero-pads h at sequence starts: first 2 halo
                    # cols must be 0, not the bias the Identity-copy wrote.
                    # (on DVE: any GpSimd op locks the shared SBUF port pair)
                    if i % PER_SEQ == 0:
                        nc.vector.memset(h_sb[:, 0:2], 0.0)
                    tiles.append(h_sb)
            h_tiles[i] = tiles

        def halo_fill(i):
            """fill right-halo cols of chunk i's h tiles."""
            cur = h_tiles[i]
            nxt = h_tiles.get(i + 1)
            for g in range(GRP):
                for ci in range(2):
                    h_sb = cur[2 * g + ci]
                    if i % PER_SEQ == PER_SEQ - 1:
                        # tokens L-2, L-1 are in no chunk's main region; their
                        # h (host-computed) + 2 zero-pad cols come via edgeS
                        nc.vector.tensor_copy(h_sb[:, T:TH],
                                              edge_sb[:, i // PER_SEQ, ci, g, :])
                    else:
                        nc.vector.tensor_copy(h_sb[:, T:TH],
                                              nxt[2 * g + ci][:, 0:4])

        def conv_main(i, c0, c1):
            """conv MAC chains + swiglu for chunk i, output cols [c0, c1)."""
            cur = h_tiles[i]
            if i not in hact_t:
                hact = hactpool.tile([128, GRP, T], BF16, tag="hact")
                hact_t[i] = hact
            hact = hact_t[i]
            w = c1 - c0
            accs = {}
            for g in range(GRP):
                for ci, (tapw, tapb) in enumerate(((cgw_sb, None),
                                                   (cuw_sb, cub_sb))):
                    acc = accpool.tile([128, w], BF16, tag="acc")
                    h_sb = cur[2 * g + ci]
                    if tapb is None:
                        nc.vector.tensor_scalar(acc[:], h_sb[:, c0:c1],
                                                tapw[:, g, 0:1], None, ALU.mult)
                    else:
                        nc.vector.tensor_scalar(acc[:], h_sb[:, c0:c1],
                                                tapw[:, g, 0:1], tapb[:, g:g + 1],
                                                ALU.mult, ALU.add)
                    accs[(g, ci)] = acc
            # MAC chains: ACT muls for 'a' slots can all start immediately;
            # the serial part per chain is stt / tt adds on DVE
            for g in range(GRP):
                for ci, tapw in enumerate((cgw_sb, cuw_sb)):
                    h_sb = cur[2 * g + ci]
                    acc = accs[(g, ci)]
                    slots = CHAIN_SLOTS[(g, ci)]
                    tmps = {}
                    for j in range(1, K):
                        if slots[j - 1] == "a":
                            tmp = tmppool.tile([128, w], BF16, tag="tmp")
                            nc.scalar.mul(tmp[:], h_sb[:, c0 + j:c1 + j],
                                          tapw[:, g, j:j + 1])
                            tmps[j] = tmp
                    for j in range(1, K):
                        if slots[j - 1] == "v":
                            nc.vector.scalar_tensor_tensor(
                                acc[:], h_sb[:, c0 + j:c1 + j],
                                tapw[:, g, j:j + 1], acc[:], ALU.mult, ALU.add)
                        else:
                            nc.vector.tensor_tensor(acc[:], tmps[j][:], acc[:],
                                                    ALU.add)
            # swiglu
            for g in range(GRP):
                gact = gactpool.tile([128, w], BF16, tag="gact")
                nc.scalar.activation(gact[:], accs[(g, 0)][:], AF.Silu,
                                     bias=cgb_sb[:, g:g + 1])
                nc.vector.tensor_tensor(hact[:, g, c0:c1], accs[(g, 1)][:],
                                        gact[:], ALU.mult)

        def down(i, c0, c1):
            """down matmul + evac + store for chunk i, cols [c0, c1)."""
            hact = hact_t[i]
            tn0, tnn = c0 // 512, (c1 - c0) // 512
            out_sb = outpool.tile([128, MSUB, (c1 - c0)], BF16, tag="out")
            for ms in range(MSUB):
                dn_ps = ps_dn.tile([128, tnn, 512], F32, tag="dn")
                for tn in range(tnn):
                    for g in range(GRP):
                        nc.tensor.matmul(
                            dn_ps[:, tn, :],
                            wd_sb[:, g, ms * 128:(ms + 1) * 128],
                            hact[:, g, (tn0 + tn) * 512:(tn0 + tn + 1) * 512],
                            start=(g == 0), stop=(g == GRP - 1))
                nc.scalar.copy(out_sb[:, ms, :], dn_ps[:])
            nc.sync.dma_start(
                yT.rearrange("(ms p) t -> p ms t", p=128)
                [:, :, i * T + c0:i * T + c1],
                out_sb[:])

        dma_x(0)
        for sb, dr in late_consts:
            nc.sync.dma_start(sb[:], dr)
        LAST = NCH - 1
        for i in range(NCH + 2):
            if i + 1 < NCH:
                dma_x(i + 1)
            if 0 <= i - 2 < LAST:
                down(i - 2, 0, T)
            if i < NCH:
                produce(i)
            if 0 <= i - 1 < NCH:
                halo_fill(i - 1)
                if i - 1 < LAST:
                    conv_main(i - 1, 0, T)
        # drain: split the last chunk in halves so its down matmuls overlap
        # the second half's conv instead of waiting for the whole chunk
        conv_main(LAST, 0, T // 2)
        down(LAST, 0, T // 2)
        conv_main(LAST, T // 2, T)
        down(LAST, T // 2, T)

    nc.compile()
    return nc


def _prep_inputs(x, Wg, bgv, Wu, buv, convg_w, convg_b, convu_w, convu_b, Wd):
    """Host-side shard/layout. Returns list of per-core in_maps."""
    x = np.ascontiguousarray(x, np.float32)
    # padded transpose: [B, D, L+4] with zero halo at sequence edges; chunk j
    # of a sequence is cols [T*j, T*j+T) = tokens [T*j-2, T*j+T-2)
    xp = np.zeros((B, D, L + 4), np.float32)
    xp[:, :, 2:L + 2] = x.transpose(0, 2, 1)
    blocks = np.stack([xp[:, :, T * j:T * j + T] for j in range(PER_SEQ)], axis=1)
    xTc = np.ascontiguousarray(
        blocks.reshape(NCH, KSUB, 128, T).transpose(0, 2, 1, 3)).astype(
            ml_dtypes.bfloat16)

    def colsplit(w, c):      # [D, F] -> per-core [128, KSUB, FS] bf16
        s = w[:, c * FS:(c + 1) * FS]
        return np.ascontiguousarray(
            s.reshape(KSUB, 128, FS).transpose(1, 0, 2)).astype(ml_dtypes.bfloat16)

    def vecsplit(v, c):      # [F] -> [128, GRP]
        return np.ascontiguousarray(v[c * FS:(c + 1) * FS].reshape(GRP, 128).T)

    xe = np.asarray(x, np.float32)[:, L - 2:L, :]          # [B, 2, D]
    in_maps = []
    for c in range(NCORES):
        sl = slice(c * FS, (c + 1) * FS)
        he_g = xe @ np.asarray(Wg, np.float32)[:, sl] + np.asarray(bgv, np.float32)[sl]
        he_u = xe @ np.asarray(Wu, np.float32)[:, sl] + np.asarray(buv, np.float32)[sl]
        edgeS = np.zeros((128, B, 2, GRP, 4), np.float32)
        for s in range(B):
            for ci, he in enumerate((he_g, he_u)):
                # [2, FS] -> [128, GRP, 2] at halo cols 0,1 (tokens L-2, L-1)
                v = he[s].reshape(2, GRP, 128).transpose(2, 1, 0)
                edgeS[:, s, ci, :, 0:2] = v
        edgeS = edgeS.astype(ml_dtypes.bfloat16)
        wdS = Wd[c * FS:(c + 1) * FS, :]
        in_maps.append({
            "edgeS": edgeS,
            "xTc": xTc,
            "wgS": colsplit(np.asarray(Wg, np.float32), c),
            "wuS": colsplit(np.asarray(Wu, np.float32), c),
            "wdS": np.ascontiguousarray(
                np.asarray(wdS, np.float32).reshape(GRP, 128, D).transpose(1, 0, 2)
                .astype(ml_dtypes.bfloat16)),
            "bgS": vecsplit(np.asarray(bgv, np.float32), c),
            "buS": vecsplit(np.asarray(buv, np.float32), c),
            "cgwS": np.ascontiguousarray(
                np.asarray(convg_w, np.float32)[c * FS:(c + 1) * FS, 0, :]
                .reshape(GRP, 128, K).transpose(1, 0, 2)),
            "cuwS": np.ascontiguousarray(
                np.asarray(convu_w, np.float32)[c * FS:(c + 1) * FS, 0, :]
                .reshape(GRP, 128, K).transpose(1, 0, 2)),
            "cgbS": vecsplit(np.asarray(convg_b, np.float32), c),
            "cubS": vecsplit(np.asarray(convu_b, np.float32), c),
        })
    return in_maps


def run_on_cores(in_maps, **kwargs):
    if "nc" not in _cache:
        _cache["nc"] = _build_program()
    return run_bass_kernel_spmd(_cache["nc"], in_maps,
                                core_ids=list(range(NCORES)), **kwargs)


def kernel(x, Wg, bg, Wu, bu, convg_w, convg_b, convu_w, convu_b, Wd, bd):
    in_maps = _prep_inputs(x, Wg, bg, Wu, bu, convg_w, convg_b,
                           convu_w, convu_b, Wd)
    res = run_on_cores(in_maps)
    acc = np.zeros((D, B * L), np.float64)
    for r in res.results:
        acc += np.asarray(r["yT"], np.float64)
    acc += np.asarray(bd, np.float64)[:, None]
    return np.ascontiguousarray(acc.T.reshape(B, L, D)).astype(np.float32)
